# revision 33
# baseline (speedup 1.0000x reference)
"""Trainium2 Bass kernel for nn_DCTCompression (8x8 block DCT + zigzag + quant).

Input : x (32, 3, 512, 512) f32
Output: (32, 192, 64, 64) f32

out[b, c*64 + z[a*8+u], hb, wb] =
    (sum_{y,x} x[b,c,hb*8+y,wb*8+x] * C8[a,y] * C8[u,x]) * NORM[a,u] * QVEC[c*64+z[a*8+u]]

Strategy (pure data parallel over batch, 4 images / core, 12 (b,c) planes):
  h-blocks are tiled INTERLEAVED: tile-row g holds blocks hb = hbP*4 + g
  (hbP = 0..15 on partitions after the transforms). Per 128x128 tile:
    MM1: P1[w, ha] = Xtile^T @ D      (D block-diag C8, cols a*16+hbP)
    MM2: P2[ha, wu] = P1^T @ D        (cols u*16+wbL)
    evict (DVE): F[...] = P2 * S_c    (S folds NORM * zigzagged quant scale)
  F layout [p=a*16+hbP][bl, c, u, g, tc, wbL] makes each zigzag output channel
  one 3-dim affine DMA covering a whole batch group, with 1KB-contiguous runs
  on both SBUF and DRAM sides.

The per-engine instruction streams are software-pipelined (MM1s run one
tile-row ahead of MM2s; ACT copies P1 PSUM->SBUF, DVE does the scaled
evictions) so that every instruction needs at most ONE semaphore wait --
this toolchain's codegen rejects instructions with 2+ sync waits.  Forced-dep
"observer" reg_movs plus the _strip_covered_waits post-pass enforce that
invariant; one-shot input tiles keep the loads wait-free.
"""

import numpy as np

import concourse.bass as bass
import concourse.mybir as mybir
import concourse.tile as tile
from concourse.bass_utils import run_bass_kernel_spmd
from concourse.tile_rust import add_dep_helper

F32 = mybir.dt.float32
F16 = mybir.dt.float16
N_CORES = 8
B, C, H, W = 32, 3, 512, 512
PLANES_PER_CORE = (B // N_CORES) * C  # 12
N_GROUPS = 1                          # one group: minimizes out-DMA count (HWDGE issue is scarce)

MM_MODE = "f16"  # "f16": ~4e-4 rel err at the memory-bandwidth floor; "f32": exact, ~30% slower (PE-bound)


# ---------------------------------------------------------------- constants
def _zigzag(n):
    p = np.zeros((n, n), dtype=np.int64)
    tri = lambda v: v * (v + 1) // 2
    for y in range(n):
        for x in range(y % 2, n - y, 2):
            p[y, x] = tri(x + y + 1) - x - 1
    for y in range(n):
        for x in range((y + 1) % 2, n - y, 2):
            p[y, x] = tri(x + y + 1) - y - 1
    for y in range(n - 1, -1, -1):
        for x in range(n - 1, -1 + (n - y), -1):
            p[y, x] = n * n - 1 - p[n - y - 1, n - x - 1]
    return p.T.copy()


def host_constants():
    PI = 3.1415
    N = 8
    t = np.arange(N, dtype=np.float64)
    C8 = np.cos(np.outer((t + 0.5) * PI, t / N)).astype(np.float32)

    norm = np.ones((N, N), dtype=np.float32)
    norm[:, 0] = np.sqrt(2.0) / 2
    norm[0, :] = np.sqrt(2.0) / 2
    norm /= 4.0

    T_LUMA = np.array(
        [[16, 11, 10, 16, 24, 40, 51, 61], [12, 12, 14, 19, 26, 58, 60, 55],
         [14, 13, 16, 24, 40, 57, 69, 56], [14, 17, 22, 29, 51, 87, 80, 62],
         [18, 22, 37, 56, 68, 109, 103, 77], [24, 35, 55, 64, 81, 104, 113, 92],
         [49, 64, 78, 87, 103, 121, 120, 101], [72, 92, 95, 98, 112, 100, 103, 99]],
        dtype=np.float32)
    Q_CHROMA = np.array(
        [[17, 18, 24, 47, 99, 99, 99, 99], [18, 21, 26, 66, 99, 99, 99, 99],
         [24, 26, 56, 99, 99, 99, 99, 99], [47, 66, 99, 99, 99, 99, 99, 99],
         [99, 99, 99, 99, 99, 99, 99, 99], [99, 99, 99, 99, 99, 99, 99, 99],
         [99, 99, 99, 99, 99, 99, 99, 99], [99, 99, 99, 99, 99, 99, 99, 99]],
        dtype=np.float32)
    Q = 50
    s = 5000.0 / Q if Q < 50 else 200 - 2 * Q
    Q_luma = np.floor((s * T_LUMA + 50) / 100)
    qvec = 1.0 / np.concatenate(
        [Q_luma.ravel(), Q_CHROMA.ravel(), Q_CHROMA.ravel()]).astype(np.float32)
    z = _zigzag(N).ravel()

    # D [128, 256]: D[g*8+t, f*16+g] = C8[f, t]; cols 128..255 are zero padding
    # (used only by the fp32r junk-padded matmuls).
    D = np.zeros((128, 256), dtype=np.float32)
    for g in range(16):
        D[g * 8:(g + 1) * 8, g:128:16] = C8.T
    # S [3, 128, 128]: S[c, a*16+hbP, u*16+wbL] = norm[a,u] * qvec[c*64+z[a*8+u]]
    S = np.zeros((3, 128, 128), dtype=np.float32)
    for c in range(3):
        for a in range(8):
            for u in range(8):
                S[c, a * 16:(a + 1) * 16, u * 16:(u + 1) * 16] = (
                    norm[a, u] * qvec[c * 64 + z[a * 8 + u]])
    return D, S, z


# ---------------------------------------------------------------- program
_ENGINE_SEM = {"PE": "PE_", "DVE": "DVE_", "Activation": "Activation_"}


def _strip_covered_waits(nc):
    """Two legal wait reductions, needed because this toolchain's codegen
    rejects instructions carrying 2+ sync waits:
      1. drop waits an earlier same-engine instruction already performed
         (engine streams execute in order, so a repeated `sem >= v'` with
         v' <= v is a no-op);
      2. drop a compute instruction's wait on its OWN engine's completion
         semaphore (PE/DVE/ACT retire strictly in order, so ordering vs an
         earlier same-engine instruction is implied by program order; only
         emitted by Tile for same-engine slot WAW, where in-order writeback
         already guarantees the overwrite order)."""
    fn = nc.m.functions[0]
    for bb in fn.blocks:
        covered = {}  # (engine, sem id) -> max waited value
        for inst in bb.instructions:
            si = inst.sync_info
            if si is None or not si.on_wait:
                continue
            eng = inst.engine
            kind = type(inst).__name__
            is_compute = kind in ("InstMatmult", "InstLdweights",
                                  "InstTensorCopy", "InstTensorTensor",
                                  "InstActivation", "InstTensorScalarPtr",
                                  "InstMemset", "InstTensorReduce")
            own_prefix = _ENGINE_SEM.get(getattr(eng, "name", str(eng)), None)
            waits = list(si.on_wait)
            if len(waits) > 1:
                keep = []
                for w in waits:
                    if (w.wait_mode == "sem-ge-imm"
                            and covered.get((eng, w.id), -1) >= w.wait_value):
                        continue
                    if (is_compute and own_prefix
                            and str(w.ant_name).startswith(own_prefix)):
                        continue
                    keep.append(w)
                if not keep:
                    keep = [waits[-1]]
                if len(keep) != len(waits):
                    inst.sync_info = mybir.SyncInfo(
                        on_wait=keep, on_update=list(si.on_update))
                waits = keep
            for w in waits:
                if w.wait_mode == "sem-ge-imm":
                    key = (eng, w.id)
                    if covered.get(key, -1) < w.wait_value:
                        covered[key] = w.wait_value


class _TC(tile.TileContext):
    """TileContext whose kernel-tail drain splits its semaphore waits across
    a chain of single-wait drains (this toolchain's codegen rejects any
    instruction carrying 2+ sync waits)."""

    def _drain_and_barrier(self, tick_clock, wait_clock):
        from concourse.vector_clock import ScopedClock

        d0 = self.nc.sync.drain()
        wait_clock.add_sem_waits(
            d0.ins, ScopedClock({None: tick_clock.global_clock}))
        si = d0.ins.sync_info
        if si is not None and len(si.on_wait) > 1:
            waits = list(si.on_wait)
            d0.ins.sync_info = mybir.SyncInfo(
                on_wait=[waits[0]], on_update=list(si.on_update))
            for w in waits[1:]:
                dn = self.nc.sync.drain()
                dn.ins.sync_info = mybir.SyncInfo(on_wait=[w], on_update=[])

        self.nc.all_engine_barrier()
        assert self.sems is not None
        popped = self.nc._tile_sem_poison_stack.pop()
        assert popped is self._sem_poison
        self.nc.clear_and_free_semaphores(list(self.sems.allocated().values()))
        self.nc.all_engine_barrier()


class _Row:
    """Per-tile-row pipeline state."""

    def __init__(self, grp, bl, c, g):
        self.grp, self.bl, self.c, self.g = grp, bl, c, g
        self.T = None
        self.P1ps = None
        self.P1sb = None
        self.P2ps = None
        self.mm1_last = None


def build_nc(mm_mode=MM_MODE, reps=1):
    assert reps == 1, "F-slot reuse across reps would need 2+ waits per evict"
    D_np, S_np, z = host_constants()
    nc = bass.Bass()

    x_d = nc.declare_dram_parameter("x", [PLANES_PER_CORE, H, W], F32, isOutput=False)
    d_d = nc.declare_dram_parameter("dconst", [128, 256], F32, isOutput=False)
    d16_d = nc.declare_dram_parameter("dconst16", [128, 128], F16, isOutput=False)
    s_d = nc.declare_dram_parameter("sconst", [3, 128, 128], F32, isOutput=False)
    out_d = nc.declare_dram_parameter("out", [B // N_CORES, C * 64, 64, 64], F32,
                                      isOutput=True)

    # out viewed as [k'(64), hbP(16), (b c)(12), (hbF w)(256)]
    out_view = out_d.rearrange("b (c k) (hp f) w -> k hp (b c) (f w)", c=3, hp=16)

    bpg = (B // N_CORES) // N_GROUPS  # b's per group (2)
    tdt = F16 if mm_mode == "f16" else F32
    p1w = 512

    rows = [_Row(grp, bl, c, g)
            for grp in range(N_GROUPS)
            for bl in range(bpg)
            for c in range(3)
            for g in range(4)]
    R = len(rows)

    with _TC(nc) as tc:
        with (
            tc.tile_pool(name="consts", bufs=1) as consts,
            tc.tile_pool(name="tin", bufs=PLANES_PER_CORE) as tpool,
            tc.tile_pool(name="p1sb", bufs=4) as midpool,
            tc.tile_pool(name="fbuf", bufs=N_GROUPS) as fpool,
            tc.tile_pool(name="psum1", bufs=3, space="PSUM") as psum1,
            tc.tile_pool(name="psum2", bufs=3, space="PSUM") as psum2,
        ):
            Dsb = consts.tile([128, 256], F32)
            dsb_load = nc.gpsimd.dma_start(out=Dsb, in_=d_d[:, :])
            Dsb16 = consts.tile([128, 128], F16)
            if mm_mode == "f16":
                nc.gpsimd.dma_start(out=Dsb16, in_=d16_d[:, :])
            Ssb = consts.tile([128, 3, 128], F32)
            nc.gpsimd.dma_start(out=Ssb, in_=s_d.rearrange("c p f -> p c f"))
            scratch = consts.tile([128, 8], F32)
            pe, dve, act = _Chain(), _Chain(), _Chain()
            pe_obs_reg = nc.tensor.alloc_register(name="pe_obs")
            act_obs_reg = nc.scalar.alloc_register(name="act_obs")
            last_evict_inst = [None]
            m0 = nc.tensor.reg_mov(pe_obs_reg, 0)
            add_dep_helper(m0.ins, dsb_load.ins, sync=True,
                           reason="PE observes const load")
            pe(m0)
            # pre-touch Ssb so the first evict doesn't carry the const-load wait
            dve(nc.vector.tensor_copy(scratch[0:1, 0:1], Ssb[0:1, 0, 0:1]))

            Dmm = Dsb16 if mm_mode == "f16" else Dsb
            Fs = [fpool.tile([128, bpg, 3, 8, 4, 4, 16], F32, name=f"F{gi}",
                             tag="F")
                  for gi in range(N_GROUPS)]
            ndma = 0
            last_evict = [None] * N_GROUPS
            obs_regs = {0: nc.sync.alloc_register(name="obs_sp"),
                        1: nc.scalar.alloc_register(name="obs_act"),
                        2: nc.gpsimd.alloc_register(name="obs_pl")}

            for i in range(R + 1):
                # ---- stage A(i): input loads + acquire P1ps + MM1 x4
                if i < R:
                    r = rows[i]
                    if r.g == 0:
                        plane = (r.grp * bpg + r.bl) * 3 + r.c
                        # h = hbP*32 + g*8 + y ; tile g: p = hbP*8 + y
                        xv = x_d[plane].rearrange("(hp g y) w -> g hp y w",
                                                  g=4, y=8)
                        # one-shot tile per plane: MM reads never hit a
                        # reused slot
                        T = tpool.tile([128, 4, 512], tdt, name=f"Tp{plane}",
                                       tag="T")
                        for g in range(4):
                            if mm_mode == "f16" or g % 2 == 0:
                                eng = nc.gpsimd
                            else:
                                eng = nc.sync if g == 1 else nc.scalar
                            eng.dma_start(out=T[:, g], in_=xv[g])
                        r.T = T
                    else:
                        r.T = rows[i - 1].T
                    r.P1ps = psum1.tile([128, p1w], F32)
                    r.mm1_last = _mm_group(
                        nc, pe, mm_mode, r.P1ps,
                        lambda t, rr=r: rr.T[:, rr.g, t * 128:(t + 1) * 128],
                        Dmm, Dsb)

                # ---- stage B(i-1): acquire P2ps + MM2 x4, then evict on DVE
                if i >= 1:
                    r = rows[i - 1]
                    if last_evict_inst[0] is not None:
                        # PE observes the evict tick so the P2ps acquirer's
                        # slot-release wait strips to a covered duplicate
                        m = nc.tensor.reg_mov(pe_obs_reg, 0)
                        add_dep_helper(m.ins, last_evict_inst[0], sync=True,
                                       reason="PE observes evicts")
                        pe(m)
                    r.P2ps = psum2.tile([128, p1w], F32)
                    _mm_group(nc, pe, mm_mode, r.P2ps,
                              lambda t, rr=r: rr.P1sb[:, t * 128:(t + 1) * 128],
                              Dmm, Dsb)

                    Fpc = Fs[r.grp][:, r.bl, r.c]
                    dst = Fpc[:, :, r.g, :, :]  # [p, u, tc, w]
                    Sv = Ssb[:, r.c].rearrange("p (u w) -> p u w", u=8)
                    sv = Sv[:, :, None, :].to_broadcast([128, 8, 4, 16])
                    src = r.P2ps.rearrange("p (t u w) -> p u t w", t=4, u=8)
                    last_evict[r.grp] = dve(nc.vector.tensor_tensor(
                        dst, src, sv, mybir.AluOpType.mult)).ins
                    last_evict_inst[0] = last_evict[r.grp]

                # ---- copy(i) on ACT (obs reg_mov covers the fresh PE tick)
                if i < R:
                    r = rows[i]
                    m = nc.scalar.reg_mov(act_obs_reg, 0)
                    add_dep_helper(m.ins, r.mm1_last, sync=True,
                                   reason="ACT observes MM1s")
                    act(m)
                    r.P1sb = midpool.tile([128, 512],
                                          F16 if mm_mode == "f16" else F32)
                    act(nc.scalar.copy(out=r.P1sb, in_=r.P1ps))

                # ---- group flush: 64 output DMAs once a group's last evict is in
                if i >= 1:
                    r = rows[i - 1]
                    if i - 1 == (r.grp + 1) * (R // N_GROUPS) - 1:
                        Fg = Fs[r.grp]
                        rings = {0: nc.sync, 1: nc.scalar, 2: nc.gpsimd}
                        ring_prev = {}
                        for ring, engo in rings.items():
                            n = engo.reg_mov(obs_regs[ring], 0)
                            add_dep_helper(n.ins, last_evict[r.grp], sync=True,
                                           reason="ring observes group evicts")
                            ring_prev[ring] = n.ins
                        pat = [0, 1, 2, 0, 1, 2, 0, 2]  # SP/ACT/Q7 mix
                        for a in range(8):
                            for u in range(8):
                                zz = int(z[a * 8 + u])
                                srcp = Fg[a * 16:(a + 1) * 16, :, :, u, :, :, :] \
                                    .rearrange("p bl c g t w -> p (bl c) (g t w)")
                                dst = out_view[zz][
                                    :, r.grp * bpg * 3:(r.grp + 1) * bpg * 3]
                                ring = pat[ndma % 8]
                                o = rings[ring].dma_start(out=dst, in_=srcp)
                                add_dep_helper(o.ins, ring_prev[ring], sync=False,
                                               reason="ring order")
                                ring_prev[ring] = o.ins
                                ndma += 1
    _strip_covered_waits(nc)
    return nc


class _Chain:
    """Forces the scheduled per-engine instruction order to match trace
    order via explicit sync=False dependency edges."""

    def __init__(self):
        self.last = None

    def __call__(self, bass_inst):
        if self.last is not None:
            add_dep_helper(bass_inst.ins, self.last, sync=False,
                           reason="pipeline order")
        self.last = bass_inst.ins
        return bass_inst


def _mm_group(nc, pe, mm_mode, psum_tile, lhsT_of, Dmm, Dsb):
    rhs = Dmm if mm_mode == "f16" else Dmm[:, :128]
    last = None
    for t in range(4):
        last = pe(nc.tensor.matmul(psum_tile[:, t * 128:(t + 1) * 128],
                                   lhsT=lhsT_of(t), rhs=rhs, start=True,
                                   stop=True))
    return last.ins


# ---------------------------------------------------------------- entry point
_NC_CACHE = {}


def get_nc(mm_mode=MM_MODE, reps=1):
    key = (mm_mode, reps)
    if key not in _NC_CACHE:
        _NC_CACHE[key] = build_nc(mm_mode, reps)
    return _NC_CACHE[key]


def make_in_maps(x):
    D_np, S_np, _ = host_constants()
    D16 = D_np[:, :128].astype(np.float16)
    per_b = B // N_CORES
    in_maps = []
    for i in range(N_CORES):
        xs = np.ascontiguousarray(
            x[i * per_b:(i + 1) * per_b].reshape(PLANES_PER_CORE, H, W))
        in_maps.append({
            "x": xs, "dconst": D_np, "dconst16": D16, "sconst": S_np,
        })
    return in_maps


def kernel(x):
    x = np.asarray(x, dtype=np.float32)
    nc = get_nc()
    res = run_bass_kernel_spmd(nc, make_in_maps(x), list(range(N_CORES)))
    out = np.concatenate([r["out"] for r in res.results], axis=0)
    return out.astype(np.float32)


# revision 34
# speedup vs baseline: 1.0007x; 1.0007x over previous
"""Trainium2 Bass kernel for nn_DCTCompression (8x8 block DCT + zigzag + quant).

Input : x (32, 3, 512, 512) f32
Output: (32, 192, 64, 64) f32

out[b, c*64 + z[a*8+u], hb, wb] =
    (sum_{y,x} x[b,c,hb*8+y,wb*8+x] * C8[a,y] * C8[u,x]) * NORM[a,u] * QVEC[c*64+z[a*8+u]]

Strategy (pure data parallel over batch, 4 images / core, 12 (b,c) planes):
  h-blocks are tiled INTERLEAVED: tile-row g holds blocks hb = hbP*4 + g
  (hbP = 0..15 on partitions after the transforms). Per 128x128 tile:
    MM1: P1[w, ha] = Xtile^T @ D      (D block-diag C8, cols a*16+hbP)
    MM2: P2[ha, wu] = P1^T @ D        (cols u*16+wbL)
    evict (DVE): F[...] = P2 * S_c    (S folds NORM * zigzagged quant scale)
  F layout [p=a*16+hbP][bl, c, u, g, tc, wbL] makes each zigzag output channel
  one 3-dim affine DMA covering a whole batch group, with 1KB-contiguous runs
  on both SBUF and DRAM sides.

The per-engine instruction streams are software-pipelined (MM1s run one
tile-row ahead of MM2s; ACT copies P1 PSUM->SBUF, DVE does the scaled
evictions) so that every instruction needs at most ONE semaphore wait --
this toolchain's codegen rejects instructions with 2+ sync waits.  Forced-dep
"observer" reg_movs plus the _strip_covered_waits post-pass enforce that
invariant; one-shot input tiles keep the loads wait-free.
"""

import numpy as np

import concourse.bass as bass
import concourse.mybir as mybir
import concourse.tile as tile
from concourse.bass_utils import run_bass_kernel_spmd
from concourse.tile_rust import add_dep_helper

F32 = mybir.dt.float32
F16 = mybir.dt.float16
N_CORES = 8
B, C, H, W = 32, 3, 512, 512
PLANES_PER_CORE = (B // N_CORES) * C  # 12
N_GROUPS = 1                          # one group: minimizes out-DMA count (HWDGE issue is scarce)

MM_MODE = "f16"  # "f16": ~4e-4 rel err at the memory-bandwidth floor; "f32": exact, ~30% slower (PE-bound)


# ---------------------------------------------------------------- constants
def _zigzag(n):
    p = np.zeros((n, n), dtype=np.int64)
    tri = lambda v: v * (v + 1) // 2
    for y in range(n):
        for x in range(y % 2, n - y, 2):
            p[y, x] = tri(x + y + 1) - x - 1
    for y in range(n):
        for x in range((y + 1) % 2, n - y, 2):
            p[y, x] = tri(x + y + 1) - y - 1
    for y in range(n - 1, -1, -1):
        for x in range(n - 1, -1 + (n - y), -1):
            p[y, x] = n * n - 1 - p[n - y - 1, n - x - 1]
    return p.T.copy()


def host_constants():
    PI = 3.1415
    N = 8
    t = np.arange(N, dtype=np.float64)
    C8 = np.cos(np.outer((t + 0.5) * PI, t / N)).astype(np.float32)

    norm = np.ones((N, N), dtype=np.float32)
    norm[:, 0] = np.sqrt(2.0) / 2
    norm[0, :] = np.sqrt(2.0) / 2
    norm /= 4.0

    T_LUMA = np.array(
        [[16, 11, 10, 16, 24, 40, 51, 61], [12, 12, 14, 19, 26, 58, 60, 55],
         [14, 13, 16, 24, 40, 57, 69, 56], [14, 17, 22, 29, 51, 87, 80, 62],
         [18, 22, 37, 56, 68, 109, 103, 77], [24, 35, 55, 64, 81, 104, 113, 92],
         [49, 64, 78, 87, 103, 121, 120, 101], [72, 92, 95, 98, 112, 100, 103, 99]],
        dtype=np.float32)
    Q_CHROMA = np.array(
        [[17, 18, 24, 47, 99, 99, 99, 99], [18, 21, 26, 66, 99, 99, 99, 99],
         [24, 26, 56, 99, 99, 99, 99, 99], [47, 66, 99, 99, 99, 99, 99, 99],
         [99, 99, 99, 99, 99, 99, 99, 99], [99, 99, 99, 99, 99, 99, 99, 99],
         [99, 99, 99, 99, 99, 99, 99, 99], [99, 99, 99, 99, 99, 99, 99, 99]],
        dtype=np.float32)
    Q = 50
    s = 5000.0 / Q if Q < 50 else 200 - 2 * Q
    Q_luma = np.floor((s * T_LUMA + 50) / 100)
    qvec = 1.0 / np.concatenate(
        [Q_luma.ravel(), Q_CHROMA.ravel(), Q_CHROMA.ravel()]).astype(np.float32)
    z = _zigzag(N).ravel()

    # D [128, 256]: D[g*8+t, f*16+g] = C8[f, t]; cols 128..255 are zero padding
    # (used only by the fp32r junk-padded matmuls).
    D = np.zeros((128, 256), dtype=np.float32)
    for g in range(16):
        D[g * 8:(g + 1) * 8, g:128:16] = C8.T
    # S [3, 128, 128]: S[c, a*16+hbP, u*16+wbL] = norm[a,u] * qvec[c*64+z[a*8+u]]
    S = np.zeros((3, 128, 128), dtype=np.float32)
    for c in range(3):
        for a in range(8):
            for u in range(8):
                S[c, a * 16:(a + 1) * 16, u * 16:(u + 1) * 16] = (
                    norm[a, u] * qvec[c * 64 + z[a * 8 + u]])
    return D, S, z


# ---------------------------------------------------------------- program
_ENGINE_SEM = {"PE": "PE_", "DVE": "DVE_", "Activation": "Activation_"}


def _strip_covered_waits(nc):
    """Two legal wait reductions, needed because this toolchain's codegen
    rejects instructions carrying 2+ sync waits:
      1. drop waits an earlier same-engine instruction already performed
         (engine streams execute in order, so a repeated `sem >= v'` with
         v' <= v is a no-op);
      2. drop a compute instruction's wait on its OWN engine's completion
         semaphore (PE/DVE/ACT retire strictly in order, so ordering vs an
         earlier same-engine instruction is implied by program order; only
         emitted by Tile for same-engine slot WAW, where in-order writeback
         already guarantees the overwrite order)."""
    fn = nc.m.functions[0]
    for bb in fn.blocks:
        covered = {}  # (engine, sem id) -> max waited value
        for inst in bb.instructions:
            si = inst.sync_info
            if si is None or not si.on_wait:
                continue
            eng = inst.engine
            kind = type(inst).__name__
            is_compute = kind in ("InstMatmult", "InstLdweights",
                                  "InstTensorCopy", "InstTensorTensor",
                                  "InstActivation", "InstTensorScalarPtr",
                                  "InstMemset", "InstTensorReduce")
            own_prefix = _ENGINE_SEM.get(getattr(eng, "name", str(eng)), None)
            waits = list(si.on_wait)
            if len(waits) > 1:
                keep = []
                for w in waits:
                    if (w.wait_mode == "sem-ge-imm"
                            and covered.get((eng, w.id), -1) >= w.wait_value):
                        continue
                    if (is_compute and own_prefix
                            and str(w.ant_name).startswith(own_prefix)):
                        continue
                    keep.append(w)
                if not keep:
                    keep = [waits[-1]]
                if len(keep) != len(waits):
                    inst.sync_info = mybir.SyncInfo(
                        on_wait=keep, on_update=list(si.on_update))
                waits = keep
            for w in waits:
                if w.wait_mode == "sem-ge-imm":
                    key = (eng, w.id)
                    if covered.get(key, -1) < w.wait_value:
                        covered[key] = w.wait_value


class _TC(tile.TileContext):
    """TileContext whose kernel-tail drain splits its semaphore waits across
    a chain of single-wait drains (this toolchain's codegen rejects any
    instruction carrying 2+ sync waits)."""

    def _drain_and_barrier(self, tick_clock, wait_clock):
        from concourse.vector_clock import ScopedClock

        d0 = self.nc.sync.drain()
        wait_clock.add_sem_waits(
            d0.ins, ScopedClock({None: tick_clock.global_clock}))
        si = d0.ins.sync_info
        if si is not None and len(si.on_wait) > 1:
            waits = list(si.on_wait)
            d0.ins.sync_info = mybir.SyncInfo(
                on_wait=[waits[0]], on_update=list(si.on_update))
            for w in waits[1:]:
                dn = self.nc.sync.drain()
                dn.ins.sync_info = mybir.SyncInfo(on_wait=[w], on_update=[])

        self.nc.all_engine_barrier()
        assert self.sems is not None
        popped = self.nc._tile_sem_poison_stack.pop()
        assert popped is self._sem_poison
        self.nc.clear_and_free_semaphores(list(self.sems.allocated().values()))
        self.nc.all_engine_barrier()


class _Row:
    """Per-tile-row pipeline state."""

    def __init__(self, grp, bl, c, g):
        self.grp, self.bl, self.c, self.g = grp, bl, c, g
        self.T = None
        self.P1ps = None
        self.P1sb = None
        self.P2ps = None
        self.mm1_last = None


def build_nc(mm_mode=MM_MODE, reps=1):
    assert reps == 1, "F-slot reuse across reps would need 2+ waits per evict"
    D_np, S_np, z = host_constants()
    nc = bass.Bass()

    x_d = nc.declare_dram_parameter("x", [PLANES_PER_CORE, H, W], F32, isOutput=False)
    d_d = nc.declare_dram_parameter("dconst", [128, 256], F32, isOutput=False)
    d16_d = nc.declare_dram_parameter("dconst16", [128, 128], F16, isOutput=False)
    s_d = nc.declare_dram_parameter("sconst", [3, 128, 128], F32, isOutput=False)
    out_d = nc.declare_dram_parameter("out", [B // N_CORES, C * 64, 64, 64], F32,
                                      isOutput=True)

    # out viewed as [k'(64), hbP(16), (b c)(12), (hbF w)(256)]
    out_view = out_d.rearrange("b (c k) (hp f) w -> k hp (b c) (f w)", c=3, hp=16)

    bpg = (B // N_CORES) // N_GROUPS  # b's per group (2)
    tdt = F16 if mm_mode == "f16" else F32
    p1w = 512

    rows = [_Row(grp, bl, c, g)
            for grp in range(N_GROUPS)
            for bl in range(bpg)
            for c in range(3)
            for g in range(4)]
    R = len(rows)

    with _TC(nc) as tc:
        with (
            tc.tile_pool(name="consts", bufs=1) as consts,
            tc.tile_pool(name="tin", bufs=PLANES_PER_CORE) as tpool,
            tc.tile_pool(name="p1sb", bufs=4) as midpool,
            tc.tile_pool(name="fbuf", bufs=N_GROUPS) as fpool,
            tc.tile_pool(name="psum1", bufs=3, space="PSUM") as psum1,
            tc.tile_pool(name="psum2", bufs=3, space="PSUM") as psum2,
        ):
            Dsb = consts.tile([128, 256], F32)
            dsb_load = nc.gpsimd.dma_start(out=Dsb, in_=d_d[:, :])
            Dsb16 = consts.tile([128, 128], F16)
            if mm_mode == "f16":
                nc.gpsimd.dma_start(out=Dsb16, in_=d16_d[:, :])
            Ssb = consts.tile([128, 3, 128], F32)
            nc.gpsimd.dma_start(out=Ssb, in_=s_d.rearrange("c p f -> p c f"))
            scratch = consts.tile([128, 8], F32)
            pe, dve, act = _Chain(), _Chain(), _Chain()
            pe_obs_reg = nc.tensor.alloc_register(name="pe_obs")
            act_obs_reg = nc.scalar.alloc_register(name="act_obs")
            last_evict_inst = [None]
            m0 = nc.tensor.reg_mov(pe_obs_reg, 0)
            add_dep_helper(m0.ins, dsb_load.ins, sync=True,
                           reason="PE observes const load")
            pe(m0)
            # pre-touch Ssb so the first evict doesn't carry the const-load wait
            dve(nc.vector.tensor_copy(scratch[0:1, 0:1], Ssb[0:1, 0, 0:1]))

            Dmm = Dsb16 if mm_mode == "f16" else Dsb
            Fs = [fpool.tile([128, bpg, 3, 8, 4, 4, 16], F32, name=f"F{gi}",
                             tag="F")
                  for gi in range(N_GROUPS)]
            ndma = 0
            last_evict = [None] * N_GROUPS
            obs_regs = {0: nc.sync.alloc_register(name="obs_sp"),
                        1: nc.scalar.alloc_register(name="obs_act"),
                        2: nc.gpsimd.alloc_register(name="obs_pl")}

            for i in range(R + 1):
                # ---- stage A(i): input loads + acquire P1ps + MM1 x4
                if i < R:
                    r = rows[i]
                    if r.g == 0:
                        plane = (r.grp * bpg + r.bl) * 3 + r.c
                        # h = hbP*32 + g*8 + y ; tile g: p = hbP*8 + y
                        xv = x_d[plane].rearrange("(hp g y) w -> g hp y w",
                                                  g=4, y=8)
                        # one-shot tile per plane: MM reads never hit a
                        # reused slot
                        T = tpool.tile([128, 4, 512], tdt, name=f"Tp{plane}",
                                       tag="T")
                        for g in range(4):
                            if mm_mode == "f16" or g % 2 == 0:
                                eng = nc.gpsimd
                            else:
                                eng = nc.sync if g == 1 else nc.scalar
                            eng.dma_start(out=T[:, g], in_=xv[g])
                        r.T = T
                    else:
                        r.T = rows[i - 1].T
                    r.P1ps = psum1.tile([128, p1w], F32)
                    r.mm1_last = _mm_group(
                        nc, pe, mm_mode, r.P1ps,
                        lambda t, rr=r: rr.T[:, rr.g, t * 128:(t + 1) * 128],
                        Dmm, Dsb)

                # ---- stage B(i-1): acquire P2ps + MM2 x4, then evict on DVE
                if i >= 1:
                    r = rows[i - 1]
                    if last_evict_inst[0] is not None:
                        # PE observes the evict tick so the P2ps acquirer's
                        # slot-release wait strips to a covered duplicate
                        m = nc.tensor.reg_mov(pe_obs_reg, 0)
                        add_dep_helper(m.ins, last_evict_inst[0], sync=True,
                                       reason="PE observes evicts")
                        pe(m)
                    r.P2ps = psum2.tile([128, p1w], F32)
                    _mm_group(nc, pe, mm_mode, r.P2ps,
                              lambda t, rr=r: rr.P1sb[:, t * 128:(t + 1) * 128],
                              Dmm, Dsb)

                    Fpc = Fs[r.grp][:, r.bl, r.c]
                    dst = Fpc[:, :, r.g, :, :]  # [p, u, tc, w]
                    Sv = Ssb[:, r.c].rearrange("p (u w) -> p u w", u=8)
                    sv = Sv[:, :, None, :].to_broadcast([128, 8, 4, 16])
                    src = r.P2ps.rearrange("p (t u w) -> p u t w", t=4, u=8)
                    last_evict[r.grp] = dve(nc.vector.tensor_tensor(
                        dst, src, sv, mybir.AluOpType.mult)).ins
                    last_evict_inst[0] = last_evict[r.grp]

                # ---- copy(i) on ACT; its {PE} wait is its only one (the
                # ACT-own slot-WAW wait is dropped by _strip_covered_waits)
                if i < R:
                    r = rows[i]
                    r.P1sb = midpool.tile([128, 512],
                                          F16 if mm_mode == "f16" else F32)
                    act(nc.scalar.copy(out=r.P1sb, in_=r.P1ps))

                # ---- group flush: 64 output DMAs once a group's last evict is in
                if i >= 1:
                    r = rows[i - 1]
                    if i - 1 == (r.grp + 1) * (R // N_GROUPS) - 1:
                        Fg = Fs[r.grp]
                        rings = {0: nc.sync, 1: nc.scalar, 2: nc.gpsimd}
                        ring_prev = {}
                        for ring, engo in rings.items():
                            n = engo.reg_mov(obs_regs[ring], 0)
                            add_dep_helper(n.ins, last_evict[r.grp], sync=True,
                                           reason="ring observes group evicts")
                            ring_prev[ring] = n.ins
                        pat = [0, 1, 2, 0, 1, 2, 0, 2]  # SP/ACT/Q7 mix
                        for a in range(8):
                            for u in range(8):
                                zz = int(z[a * 8 + u])
                                srcp = Fg[a * 16:(a + 1) * 16, :, :, u, :, :, :] \
                                    .rearrange("p bl c g t w -> p (bl c) (g t w)")
                                dst = out_view[zz][
                                    :, r.grp * bpg * 3:(r.grp + 1) * bpg * 3]
                                ring = pat[ndma % 8]
                                o = rings[ring].dma_start(out=dst, in_=srcp)
                                add_dep_helper(o.ins, ring_prev[ring], sync=False,
                                               reason="ring order")
                                ring_prev[ring] = o.ins
                                ndma += 1
    _strip_covered_waits(nc)
    return nc


class _Chain:
    """Forces the scheduled per-engine instruction order to match trace
    order via explicit sync=False dependency edges."""

    def __init__(self):
        self.last = None

    def __call__(self, bass_inst):
        if self.last is not None:
            add_dep_helper(bass_inst.ins, self.last, sync=False,
                           reason="pipeline order")
        self.last = bass_inst.ins
        return bass_inst


def _mm_group(nc, pe, mm_mode, psum_tile, lhsT_of, Dmm, Dsb):
    rhs = Dmm if mm_mode == "f16" else Dmm[:, :128]
    last = None
    for t in range(4):
        last = pe(nc.tensor.matmul(psum_tile[:, t * 128:(t + 1) * 128],
                                   lhsT=lhsT_of(t), rhs=rhs, start=True,
                                   stop=True))
    return last.ins


# ---------------------------------------------------------------- entry point
_NC_CACHE = {}


def get_nc(mm_mode=MM_MODE, reps=1):
    key = (mm_mode, reps)
    if key not in _NC_CACHE:
        _NC_CACHE[key] = build_nc(mm_mode, reps)
    return _NC_CACHE[key]


def make_in_maps(x):
    D_np, S_np, _ = host_constants()
    D16 = D_np[:, :128].astype(np.float16)
    per_b = B // N_CORES
    in_maps = []
    for i in range(N_CORES):
        xs = np.ascontiguousarray(
            x[i * per_b:(i + 1) * per_b].reshape(PLANES_PER_CORE, H, W))
        in_maps.append({
            "x": xs, "dconst": D_np, "dconst16": D16, "sconst": S_np,
        })
    return in_maps


def kernel(x):
    x = np.asarray(x, dtype=np.float32)
    nc = get_nc()
    res = run_bass_kernel_spmd(nc, make_in_maps(x), list(range(N_CORES)))
    out = np.concatenate([r["out"] for r in res.results], axis=0)
    return out.astype(np.float32)
